# revision 7
# baseline (speedup 1.0000x reference)
"""DeepseekMoE layer on 8 TRN2 NeuronCores — expert-parallel Bass/Tile kernel.

Strategy (self-contained, shapes hardcoded for this problem):
  H=2048, T=2048 tokens, E=16 experts, top-6, I=1408, shared IS=2816.

  Sharding (done on host inside kernel(), per the full-input contract):
    - Router (softmax + top-6) computed on host in fp32 -> per-expert token
      lists (the "all-to-all dispatch" decision).
    - Core c owns experts 2c, 2c+1: receives w1/w2 transposed for those
      experts plus the gathered+transposed x columns of the tokens routed to
      them (capacity-padded to C), and the routing weights.
    - Shared expert is sharded over its intermediate dim: core c owns
      rows [352c, 352c+352) (padded to 384 = 3*128) of the shared MLP.
    - Each core returns per-expert outputs [C, H] (pre-scaled by routing
      weights) and a dense shared partial [T, H]; host scatter-adds.

  On-device per expert e:
    s1:  gate_up.T[o, t] = sum_h w1t[h, o] * xsel[h, t]   (fp32r matmuls)
         silu fused into PSUM eviction; up-eviction is an in-place multiply
         -> act.T [i, t] in SBUF (fp32r)
    s2:  y[t, h] = sum_i act.T[i, t] * w2t[i, h], eviction fused with
         per-token routing-weight scale (ACT Copy, scale AP).
  Shared expert: identical structure over all T (two 1024-token halves).
"""

import os
import sys

sys.path.insert(0, "/opt/trn_rl_repo")

import numpy as np

import concourse.bass as bass  # noqa: F401
import concourse.tile as tile
from concourse import bacc, mybir
from concourse.bass_utils import run_bass_kernel_spmd

H = 2048
T = 2048
E = 16
TOPK = 6
I2 = 2816  # 2*I
I = 1408
ISH = 2816  # shared intermediate (per gate/up half)
NCORES = 8
C = 1024  # per-expert token capacity (avg load 768, ~11 sigma headroom)
SSL = 352  # shared-intermediate slice per core
SSLP = 384  # padded to 3*128

F32 = mybir.dt.float32
F32R = mybir.dt.float32r
AF = mybir.ActivationFunctionType

_compiled = {}
last_result = None  # BassKernelResults of the most recent run (for profiling)


def _emit_mlp_block(nc, pools, *, w1t_ap, x_tile, act_tile, w2t_ap, out_ap,
                    out_row0, n_gate_ot, s2_k, cw_tile):
    """One MLP block: gate/up matmul + silu*up, then down-proj matmul.

    w1t_ap:  DRAM [H, 2*n_gate_ot*128] (gate cols then up cols)
    x_tile:  SBUF [128, 16, 1024] fp32r (x.T columns for this block)
    act_tile: SBUF [128, >=s2_k, 1024] fp32r (written here)
    w2t_ap:  DRAM [s2_k*128, H]
    out_ap:  DRAM output, rows [out_row0, out_row0+1024), all H cols
    n_gate_ot: number of 128-row o-tiles in the gate half
    s2_k == n_gate_ot: contraction i-tiles for stage 2
    cw_tile: SBUF [128, 8] per-token scale, or None
    """
    w1p, w2p, psp, outp = pools["w1"], pools["w2"], pools["ps"], pools["out"]
    KT = 16  # h contraction tiles
    NT = 2   # 512-wide token chunks per 1024 block
    w1t_r = w1t_ap.rearrange("(k p) o -> p k o", p=128)
    w2t_r = w2t_ap.rearrange("(k p) h -> p k h", p=128)

    # stage 1: gate_up.T tiles, silu fused, in-place up-mul
    for ot in range(2 * n_gate_ot):
        w1slab = w1p.tile([128, KT, 128], F32R, tag="w1slab")
        nc.sync.dma_start(out=w1slab[:], in_=w1t_r[:, :, ot * 128:(ot + 1) * 128])
        for tc in range(NT):
            ps = psp.tile([128, 512], F32, tag="ps")
            for k in range(KT):
                nc.tensor.matmul(
                    ps[:],
                    w1slab[:, k, :],
                    x_tile[:, k, tc * 512:(tc + 1) * 512],
                    start=(k == 0),
                    stop=(k == KT - 1),
                )
            if ot < n_gate_ot:
                nc.scalar.activation(
                    out=act_tile[:, ot, tc * 512:(tc + 1) * 512],
                    in_=ps[:],
                    func=AF.Silu,
                )
            else:
                sl = act_tile[:, ot - n_gate_ot, tc * 512:(tc + 1) * 512]
                nc.vector.tensor_mul(sl, ps[:], sl)

    # stage 2: down proj, scale fused into eviction
    for hc in range(4):
        w2slab = w2p.tile([128, s2_k, 512], F32R, tag="w2slab")
        nc.sync.dma_start(out=w2slab[:], in_=w2t_r[:, :, hc * 512:(hc + 1) * 512])
        for tt in range(8):
            ps = psp.tile([128, 512], F32, tag="ps")
            for k in range(s2_k):
                nc.tensor.matmul(
                    ps[:],
                    act_tile[:, k, tt * 128:(tt + 1) * 128],
                    w2slab[:, k, :],
                    start=(k == 0),
                    stop=(k == s2_k - 1),
                )
            ysb = outp.tile([128, 512], F32, tag="ysb")
            if cw_tile is not None:
                nc.scalar.activation(out=ysb[:], in_=ps[:], func=AF.Copy,
                                     scale=cw_tile[:, tt:tt + 1])
            else:
                nc.scalar.activation(out=ysb[:], in_=ps[:], func=AF.Copy)
            nc.sync.dma_start(
                out=out_ap[out_row0 + tt * 128: out_row0 + (tt + 1) * 128,
                           hc * 512:(hc + 1) * 512],
                in_=ysb[:],
            )


def _build(capacity):
    global C
    C = capacity
    nc = bacc.Bacc("TRN2", target_bir_lowering=False, debug=False)

    aps = {}
    for j in range(2):
        aps[f"xs{j}"] = nc.dram_tensor(f"xs{j}", [H, C], F32R, kind="ExternalInput").ap()
        aps[f"w1t{j}"] = nc.dram_tensor(f"w1t{j}", [H, I2], F32R, kind="ExternalInput").ap()
        aps[f"w2t{j}"] = nc.dram_tensor(f"w2t{j}", [I, H], F32R, kind="ExternalInput").ap()
        aps[f"cw{j}"] = nc.dram_tensor(f"cw{j}", [C], F32, kind="ExternalInput").ap()
        aps[f"y{j}"] = nc.dram_tensor(f"y{j}", [C, H], F32, kind="ExternalOutput").ap()
    aps["xt"] = nc.dram_tensor("xt", [H, T], F32R, kind="ExternalInput").ap()
    aps["sw1t"] = nc.dram_tensor("sw1t", [H, 2 * SSLP], F32R, kind="ExternalInput").ap()
    aps["sw2t"] = nc.dram_tensor("sw2t", [SSLP, H], F32R, kind="ExternalInput").ap()
    aps["ys"] = nc.dram_tensor("ys", [T, H], F32, kind="ExternalOutput").ap()

    import contextlib
    with tile.TileContext(nc) as tc, contextlib.ExitStack() as ctx:
        pools = {
            "x": ctx.enter_context(tc.tile_pool(name="x", bufs=1)),
            "w1": ctx.enter_context(tc.tile_pool(name="w1", bufs=3)),
            "w2": ctx.enter_context(tc.tile_pool(name="w2", bufs=2)),
            "act": ctx.enter_context(tc.tile_pool(name="act", bufs=1)),
            "out": ctx.enter_context(tc.tile_pool(name="out", bufs=3)),
            "ps": ctx.enter_context(tc.tile_pool(name="ps", bufs=8, space="PSUM")),
            "misc": ctx.enter_context(tc.tile_pool(name="misc", bufs=2)),
        }

        # experts (in 1024-token sub-blocks to keep SBUF tiling fixed)
        for j in range(2):
            xs_r = aps[f"xs{j}"].rearrange("(k p) t -> p k t", p=128)
            cw_r = aps[f"cw{j}"].rearrange("(n p) -> p n", p=128)
            for blk in range(C // 1024):
                x_tile = pools["x"].tile([128, 16, 1024], F32R, tag="xsel")
                nc.sync.dma_start(out=x_tile[:],
                                  in_=xs_r[:, :, blk * 1024:(blk + 1) * 1024])
                cw_tile = pools["misc"].tile([128, 8], F32, tag="cw")
                nc.sync.dma_start(out=cw_tile[:],
                                  in_=cw_r[:, blk * 8:(blk + 1) * 8])
                act_tile = pools["act"].tile([128, 11, 1024], F32R, tag="act")
                _emit_mlp_block(
                    nc, pools,
                    w1t_ap=aps[f"w1t{j}"], x_tile=x_tile, act_tile=act_tile,
                    w2t_ap=aps[f"w2t{j}"], out_ap=aps[f"y{j}"],
                    out_row0=blk * 1024,
                    n_gate_ot=11, s2_k=11, cw_tile=cw_tile,
                )

        # shared expert slice, two 1024-token halves
        xt_r = aps["xt"].rearrange("(k p) t -> p k t", p=128)
        for half in range(2):
            x_tile = pools["x"].tile([128, 16, 1024], F32R, tag="xsel")
            nc.sync.dma_start(out=x_tile[:],
                              in_=xt_r[:, :, half * 1024:(half + 1) * 1024])
            act_tile = pools["act"].tile([128, 3, 1024], F32R, tag="act")
            _emit_mlp_block(
                nc, pools,
                w1t_ap=aps["sw1t"], x_tile=x_tile, act_tile=act_tile,
                w2t_ap=aps["sw2t"], out_ap=aps["ys"], out_row0=half * 1024,
                n_gate_ot=3, s2_k=3, cw_tile=None,
            )

    nc.compile()
    return nc


def _route(xf, gate_w):
    """Host router: fp32 softmax + top-6.

    Uses jax on CPU when available so selection/weights match the jax
    reference bit-for-bit (matters only for near-exact prob ties).
    """
    try:
        import jax
        import jax.numpy as jnp

        cpu = jax.devices("cpu")[0]
        with jax.default_device(cpu):
            logits = jnp.asarray(xf) @ jnp.asarray(gate_w).T
            probs = jax.nn.softmax(logits.astype(jnp.float32), axis=-1)
            _, sel = jax.lax.top_k(probs, TOPK)
        return np.asarray(probs), np.asarray(sel)
    except Exception:
        logits = xf @ gate_w.T  # [T, E] fp32
        m = logits.max(axis=-1, keepdims=True)
        e = np.exp(logits - m, dtype=np.float32)
        probs = e / e.sum(axis=-1, keepdims=True)
        sel = np.argsort(-probs, axis=-1, kind="stable")[:, :TOPK]
        return probs, sel


def kernel(x, gate_w, w1, w2, shared_w1, shared_w2):
    x = np.asarray(x, np.float32)
    gate_w = np.asarray(gate_w, np.float32)
    w1 = np.asarray(w1, np.float32)
    w2 = np.asarray(w2, np.float32)
    shared_w1 = np.asarray(shared_w1, np.float32)
    shared_w2 = np.asarray(shared_w2, np.float32)

    B, S, Hd = x.shape
    xf = np.ascontiguousarray(x.reshape(-1, Hd))  # [T, H]

    probs, sel = _route(xf, gate_w)
    onehot = np.zeros((T, E), bool)
    onehot[np.arange(T)[:, None], sel] = True
    idx_e = [np.nonzero(onehot[:, e])[0] for e in range(E)]
    counts = np.array([len(ix) for ix in idx_e])

    cap = C
    while counts.max() > cap:
        cap *= 2
    if cap not in _compiled:
        _compiled[cap] = _build(cap)
    nc = _compiled[cap]

    xt = np.ascontiguousarray(xf.T)  # [H, T]

    in_maps = []
    for c in range(NCORES):
        m = {"xt": xt}
        for j in range(2):
            e = 2 * c + j
            ix = idx_e[e]
            xs = np.zeros((cap, H), np.float32)
            xs[: len(ix)] = xf[ix]
            m[f"xs{j}"] = np.ascontiguousarray(xs.T)
            m[f"w1t{j}"] = np.ascontiguousarray(w1[e].T)
            m[f"w2t{j}"] = np.ascontiguousarray(w2[e].T)
            cw = np.zeros(cap, np.float32)
            cw[: len(ix)] = probs[ix, e]
            m[f"cw{j}"] = cw
        sw1t = np.zeros((H, 2 * SSLP), np.float32)
        sw1t[:, :SSL] = shared_w1[SSL * c: SSL * (c + 1)].T
        sw1t[:, SSLP: SSLP + SSL] = shared_w1[ISH + SSL * c: ISH + SSL * (c + 1)].T
        m["sw1t"] = sw1t
        sw2t = np.zeros((SSLP, H), np.float32)
        sw2t[:SSL] = shared_w2[:, SSL * c: SSL * (c + 1)].T
        m["sw2t"] = sw2t
        in_maps.append(m)

    try:
        res = run_bass_kernel_spmd(nc, in_maps, list(range(NCORES)))
    except ModuleNotFoundError:
        # BASS_TRACE=1 requires the axon NTFF hook (antenv.axon_hooks),
        # absent in some containers — retry with tracing disabled.
        os.environ["BASS_NEVER_TRACE"] = "1"
        res = run_bass_kernel_spmd(nc, in_maps, list(range(NCORES)))
    global last_result
    last_result = res

    out = np.zeros((T, H), np.float32)
    for c in range(NCORES):
        out += res.results[c]["ys"]
        for j in range(2):
            e = 2 * c + j
            ix = idx_e[e]
            out[ix] += res.results[c][f"y{j}"][: len(ix)]

    return out.reshape(B, S, Hd)


# revision 32
# speedup vs baseline: 1.1198x; 1.1198x over previous
"""DeepseekMoE layer on 8 TRN2 NeuronCores — expert-parallel Bass/Tile kernel.

Strategy (self-contained, shapes hardcoded for this problem):
  H=2048, T=2048 tokens, E=16 experts, top-6, I=1408, shared IS=2816.

  Sharding (done on host inside kernel(), per the full-input contract):
    - Router (softmax + top-6) computed on host in fp32 (jax-on-CPU when
      available so near-tie selections match the jax reference bitwise)
      -> per-expert token lists (the "all-to-all dispatch" decision).
    - Core c owns experts 2c, 2c+1: receives w1/w2 transposed for those
      experts plus the gathered+transposed x columns of the tokens routed to
      them (capacity-padded to CAP), and the routing weights.
    - Shared expert is sharded over its intermediate dim: core c owns
      rows [352c, 352c+352) (padded to 384 = 3*128) of the shared MLP.
    - Each core returns per-expert outputs [CAP, H] (pre-scaled by routing
      weights) and a dense shared partial [T, H]; host scatter-adds.

  On-device per expert e (all matmuls fp32r = full PE rate, ~1.5e-4 rms):
    s1:  gate_up.T[o, t] = sum_h w1t[h, o] * xsel[h, t]
         silu fused into PSUM eviction; up-eviction is an in-place multiply
         -> act.T [i, t] in SBUF (fp32r)
    s2:  y[t, h] = sum_i act.T[i, t] * w2t[i, h], eviction fused with
         per-token routing-weight scale (ACT Copy, scale AP).
  Shared expert: identical structure over all T in 1024-token halves.
"""

import os
import sys

sys.path.insert(0, "/opt/trn_rl_repo")

import numpy as np

import concourse.bass as bass  # noqa: F401
import concourse.tile as tile
from concourse import bacc, mybir
from concourse.bass_utils import run_bass_kernel_spmd

H = 2048
T = 2048
E = 16
TOPK = 6
I2 = 2816  # 2*I
I = 1408
ISH = 2816  # shared intermediate (per gate/up half)
NCORES = 8
CAP0 = 896  # per-expert token capacity (avg load 768); grown if exceeded
SSL = 352  # shared-intermediate slice per core
SSLP = 384  # padded to 3*128

F32 = mybir.dt.float32
F32R = mybir.dt.float32r
AF = mybir.ActivationFunctionType

_compiled = {}
last_result = None  # BassKernelResults of the most recent run (for profiling)


def _nchunks(n):
    """Split n (multiple of 128) into fp32-matmul-friendly free-dim chunks:
    each <= 512 and >= 256 (fp32r runs 1 cyc/row only at N >= 256)."""
    out = []
    while n > 0:
        if n > 512:
            out.append(512)
            n -= 512
        elif n >= 256 or not out:
            out.append(n)
            n = 0
        else:  # n == 128: rebalance with previous 512 -> 384 + 256
            out[-1] -= 128
            out.append(256)
            n = 0
    return out


def _emit_s1(nc, pools, *, w1t_ap, x_tile, act_tile, ntok, n_gate_ot,
             first_slab_hipri=False):
    """Stage 1: gate_up.T tiles, silu fused into eviction, in-place up-mul.

    w1t_ap:  DRAM [H, 2*n_gate_ot*128] (gate cols then up cols)
    x_tile:  SBUF [128, 16, ntok] fp32r (x.T columns for this block)
    act_tile: SBUF [128, n_gate_ot, ntok] fp32r (written here)
    """
    w1p, psp = pools["w1"], pools["ps"]
    KT = 16  # h contraction tiles
    w1t_r = w1t_ap.rearrange("(k p) o -> p k o", p=128)
    spans = []
    t0 = 0
    for tcw in _nchunks(ntok):
        spans.append((t0, tcw))
        t0 += tcw
    tc = pools["tc"]
    for ot in range(2 * n_gate_ot):
        w1slab = w1p.tile([128, KT, 128], F32R, tag="w1slab")
        if ot == 0 and first_slab_hipri:
            with tc.high_priority():
                nc.sync.dma_start(out=w1slab[:],
                                  in_=w1t_r[:, :, ot * 128:(ot + 1) * 128])
        else:
            nc.sync.dma_start(out=w1slab[:],
                              in_=w1t_r[:, :, ot * 128:(ot + 1) * 128])
        # k outer / chunk inner: consecutive matmuls reuse the stationary
        # operand w1slab[:, k, :], amortizing its LDWEIGHTS
        pss = [psp.tile([128, 512], F32, tag="ps", name=f"ps1_{ot}_{ci}")
               for ci in range(len(spans))]
        for k in range(KT):
            for ci, (t0, tcw) in enumerate(spans):
                nc.tensor.matmul(
                    pss[ci][:, :tcw],
                    w1slab[:, k, :],
                    x_tile[:, k, t0:t0 + tcw],
                    start=(k == 0),
                    stop=(k == KT - 1),
                )
        for ci, (t0, tcw) in enumerate(spans):
            if ot < n_gate_ot:
                nc.scalar.activation(
                    out=act_tile[:, ot, t0:t0 + tcw],
                    in_=pss[ci][:, :tcw],
                    func=AF.Silu,
                )
            else:
                sl = act_tile[:, ot - n_gate_ot, t0:t0 + tcw]
                nc.vector.tensor_mul(sl, pss[ci][:, :tcw], sl)


def _emit_s2(nc, pools, *, act_tile, w2t_ap, out_ap, out_row0, ntok,
             n_gate_ot, cw_tile, cw_col0=0):
    """Stage 2: down proj, per-token scale fused into eviction.

    w2t_ap:  DRAM [n_gate_ot*128, H]
    out_ap:  DRAM output, rows [out_row0, out_row0+ntok), all H cols
    cw_tile: SBUF [128, >=cw_col0+ntok/128] per-token scale, or None
    """
    w2p, psp, outp = pools["w2"], pools["ps"], pools["out"]
    s2_k = n_gate_ot
    w2t_r = w2t_ap.rearrange("(k p) h -> p k h", p=128)
    for hc in range(4):
        w2slab = w2p.tile([128, s2_k, 512], F32R, tag="w2slab",
                          name=f"w2slab_{hc}")
        nc.sync.dma_start(out=w2slab[:],
                          in_=w2t_r[:, :, hc * 512:(hc + 1) * 512])
        for tt in range(ntok // 128):
            ps = psp.tile([128, 512], F32, tag="ps", name=f"ps2_{hc}_{tt}")
            for k in range(s2_k):
                nc.tensor.matmul(
                    ps[:],
                    act_tile[:, k, tt * 128:(tt + 1) * 128],
                    w2slab[:, k, :],
                    start=(k == 0),
                    stop=(k == s2_k - 1),
                )
            ysb = outp.tile([128, 512], F32, tag="ysb", name=f"ysb_{hc}_{tt}")
            if cw_tile is not None:
                nc.scalar.activation(
                    out=ysb[:], in_=ps[:], func=AF.Copy,
                    scale=cw_tile[:, cw_col0 + tt:cw_col0 + tt + 1])
            else:
                nc.scalar.activation(out=ysb[:], in_=ps[:], func=AF.Copy)
            nc.sync.dma_start(
                out=out_ap[out_row0 + tt * 128: out_row0 + (tt + 1) * 128,
                           hc * 512:(hc + 1) * 512],
                in_=ysb[:],
            )


def _build(cap):
    nc = bacc.Bacc("TRN2", target_bir_lowering=False, debug=False)

    aps = {}
    for j in range(2):
        aps[f"xs{j}"] = nc.dram_tensor(f"xs{j}", [H, cap], F32R, kind="ExternalInput").ap()
        aps[f"w1t{j}"] = nc.dram_tensor(f"w1t{j}", [H, I2], F32R, kind="ExternalInput").ap()
        aps[f"w2t{j}"] = nc.dram_tensor(f"w2t{j}", [I, H], F32R, kind="ExternalInput").ap()
        aps[f"cw{j}"] = nc.dram_tensor(f"cw{j}", [cap], F32, kind="ExternalInput").ap()
        aps[f"y{j}"] = nc.dram_tensor(f"y{j}", [cap, H], F32, kind="ExternalOutput").ap()
    aps["xt"] = nc.dram_tensor("xt", [H, T], F32R, kind="ExternalInput").ap()
    aps["sw1t"] = nc.dram_tensor("sw1t", [H, 2 * SSLP], F32R, kind="ExternalInput").ap()
    aps["sw2t"] = nc.dram_tensor("sw2t", [SSLP, H], F32R, kind="ExternalInput").ap()
    aps["ys"] = nc.dram_tensor("ys", [T, H], F32, kind="ExternalOutput").ap()

    # token blocks per expert (<=1024 each, multiples of 128)
    eblocks = []
    r0 = 0
    while r0 < cap:
        w = min(1024, cap - r0)
        eblocks.append((r0, w))
        r0 += w

    import contextlib
    with tile.TileContext(nc) as tc, contextlib.ExitStack() as ctx:
        pools = {
            "x": ctx.enter_context(tc.tile_pool(name="x", bufs=1)),
            "w1": ctx.enter_context(tc.tile_pool(name="w1", bufs=3)),
            "w2": ctx.enter_context(tc.tile_pool(name="w2", bufs=2)),
            "act": ctx.enter_context(tc.tile_pool(name="act", bufs=1)),
            "out": ctx.enter_context(tc.tile_pool(name="out", bufs=3)),
            "ps": ctx.enter_context(tc.tile_pool(name="ps", bufs=8, space="PSUM")),
            "misc": ctx.enter_context(tc.tile_pool(name="misc", bufs=2)),
        }

        pools["tc"] = tc
        cw_tiles = {}

        def get_cw(j):  # lazy: cw loads shouldn't precede compute-critical DMAs
            if j not in cw_tiles:
                cw_r = aps[f"cw{j}"].rearrange("(n p) -> p n", p=128)
                cw_tiles[j] = pools["misc"].tile([128, cap // 128], F32,
                                                 tag=f"cw{j}", name=f"cw{j}_t")
                nc.sync.dma_start(out=cw_tiles[j][:], in_=cw_r[:])
            return cw_tiles[j]

        # block list: expert sub-blocks then the two shared-expert halves.
        blocks = []
        for j in range(2):
            xs_r = aps[f"xs{j}"].rearrange("(k p) t -> p k t", p=128)
            for (row0, ntok) in eblocks:
                blocks.append(dict(
                    x_src=xs_r[:, :, row0:row0 + ntok], ntok=ntok, n_gate_ot=11,
                    w1t_ap=aps[f"w1t{j}"], w2t_ap=aps[f"w2t{j}"],
                    out_ap=aps[f"y{j}"], out_row0=row0,
                    cw_j=j, cw_col0=row0 // 128,
                ))
        xt_r = aps["xt"].rearrange("(k p) t -> p k t", p=128)
        for half in range(2):
            blocks.append(dict(
                x_src=xt_r[:, :, half * 1024:(half + 1) * 1024], ntok=1024,
                n_gate_ot=3, w1t_ap=aps["sw1t"], w2t_ap=aps["sw2t"],
                out_ap=aps["ys"], out_row0=half * 1024,
                cw_j=None, cw_col0=0,
            ))

        def load_x(b, split=False):
            xt_tile = pools["x"].tile([128, 16, b["ntok"]], F32R, tag="xsel")
            if split:  # first block: only chunk 0 is compute-critical
                t0 = 0
                for ci, tcw in enumerate(_nchunks(b["ntok"])):
                    if ci == 0:
                        with tc.high_priority():
                            nc.sync.dma_start(out=xt_tile[:, :, t0:t0 + tcw],
                                              in_=b["x_src"][:, :, t0:t0 + tcw])
                    else:
                        nc.sync.dma_start(out=xt_tile[:, :, t0:t0 + tcw],
                                          in_=b["x_src"][:, :, t0:t0 + tcw])
                    t0 += tcw
            else:
                with tc.high_priority():
                    nc.sync.dma_start(out=xt_tile[:], in_=b["x_src"])
            return xt_tile

        # Emit s1(n), then block n+1's x-load, then s2(n): the next x-load
        # lands ahead of s2(n)'s weight slabs in the scheduler's priority
        # order, so its (large) transfer overlaps s2(n) compute instead of
        # queueing behind it in the DGE FIFO.
        x_tiles = [load_x(blocks[0], split=True)]
        for n, b in enumerate(blocks):
            act_tile = pools["act"].tile([128, b["n_gate_ot"], b["ntok"]],
                                         F32R, tag="act")
            _emit_s1(nc, pools, w1t_ap=b["w1t_ap"], x_tile=x_tiles[n],
                     act_tile=act_tile, ntok=b["ntok"],
                     n_gate_ot=b["n_gate_ot"], first_slab_hipri=(n == 0))
            if n + 1 < len(blocks):
                x_tiles.append(load_x(blocks[n + 1]))
            _emit_s2(nc, pools, act_tile=act_tile, w2t_ap=b["w2t_ap"],
                     out_ap=b["out_ap"], out_row0=b["out_row0"],
                     ntok=b["ntok"], n_gate_ot=b["n_gate_ot"],
                     cw_tile=None if b["cw_j"] is None else get_cw(b["cw_j"]),
                     cw_col0=b["cw_col0"])

    nc.compile()
    return nc


def _route(xf, gate_w):
    """Host router: fp32 softmax + top-6.

    Uses jax on CPU when available so selection/weights match the jax
    reference bit-for-bit (matters only for near-exact prob ties).
    """
    try:
        import jax
        import jax.numpy as jnp

        cpu = jax.devices("cpu")[0]
        with jax.default_device(cpu):
            logits = jnp.asarray(xf) @ jnp.asarray(gate_w).T
            probs = jax.nn.softmax(logits.astype(jnp.float32), axis=-1)
            _, sel = jax.lax.top_k(probs, TOPK)
        return np.asarray(probs), np.asarray(sel)
    except Exception:
        logits = xf @ gate_w.T  # [T, E] fp32
        m = logits.max(axis=-1, keepdims=True)
        e = np.exp(logits - m, dtype=np.float32)
        probs = e / e.sum(axis=-1, keepdims=True)
        sel = np.argsort(-probs, axis=-1, kind="stable")[:, :TOPK]
        return probs, sel


def kernel(x, gate_w, w1, w2, shared_w1, shared_w2):
    x = np.asarray(x, np.float32)
    gate_w = np.asarray(gate_w, np.float32)
    w1 = np.asarray(w1, np.float32)
    w2 = np.asarray(w2, np.float32)
    shared_w1 = np.asarray(shared_w1, np.float32)
    shared_w2 = np.asarray(shared_w2, np.float32)

    B, S, Hd = x.shape
    xf = np.ascontiguousarray(x.reshape(-1, Hd))  # [T, H]

    probs, sel = _route(xf, gate_w)
    onehot = np.zeros((T, E), bool)
    onehot[np.arange(T)[:, None], sel] = True
    idx_e = [np.nonzero(onehot[:, e])[0] for e in range(E)]
    counts = np.array([len(ix) for ix in idx_e])

    cap = CAP0
    while counts.max() > cap:
        cap += 128
    if cap not in _compiled:
        _compiled[cap] = _build(cap)
    nc = _compiled[cap]

    xt = np.ascontiguousarray(xf.T)  # [H, T]

    in_maps = []
    for c in range(NCORES):
        m = {"xt": xt}
        for j in range(2):
            e = 2 * c + j
            ix = idx_e[e]
            xs = np.zeros((cap, H), np.float32)
            xs[: len(ix)] = xf[ix]
            m[f"xs{j}"] = np.ascontiguousarray(xs.T)
            m[f"w1t{j}"] = np.ascontiguousarray(w1[e].T)
            m[f"w2t{j}"] = np.ascontiguousarray(w2[e].T)
            cw = np.zeros(cap, np.float32)
            cw[: len(ix)] = probs[ix, e]
            m[f"cw{j}"] = cw
        sw1t = np.zeros((H, 2 * SSLP), np.float32)
        sw1t[:, :SSL] = shared_w1[SSL * c: SSL * (c + 1)].T
        sw1t[:, SSLP: SSLP + SSL] = shared_w1[ISH + SSL * c: ISH + SSL * (c + 1)].T
        m["sw1t"] = sw1t
        sw2t = np.zeros((SSLP, H), np.float32)
        sw2t[:SSL] = shared_w2[:, SSL * c: SSL * (c + 1)].T
        m["sw2t"] = sw2t
        in_maps.append(m)

    try:
        res = run_bass_kernel_spmd(nc, in_maps, list(range(NCORES)))
    except ModuleNotFoundError:
        # BASS_TRACE=1 requires the axon NTFF hook (antenv.axon_hooks),
        # absent in some containers — retry with tracing disabled.
        os.environ["BASS_NEVER_TRACE"] = "1"
        res = run_bass_kernel_spmd(nc, in_maps, list(range(NCORES)))
    global last_result
    last_result = res

    out = np.zeros((T, H), np.float32)
    for c in range(NCORES):
        out += res.results[c]["ys"]
        for j in range(2):
            e = 2 * c + j
            ix = idx_e[e]
            out[ix] += res.results[c][f"y{j}"][: len(ix)]

    return out.reshape(B, S, Hd)
